# revision 6
# baseline (speedup 1.0000x reference)
"""Trainium2 Bass kernel for nn_MultiHeadAttention_21345987461791.

Reference computation (B=4, S=4096, HID=1024, NUM_HEADS=16, HEAD_DIM=64):
    qh = (q @ Wq + bq)  -> [B,S,16,64]   (same for k, v)
    scores = einsum('bshd,bstd->bsht', qh, kh) / 8     # per-token [16,16]
    w = softmax(scores, -1)
    out = einsum('bsht,bstd->bshd', w, vh) @ Wo + bo
    LayerNorm-ish: ln_w * (out - mu) / (var_unbiased + eps) + ln_b   (NO sqrt!)

Sharding: pure data-parallel over tokens. B*S = 16384 tokens -> 2048 per core
(8 cores). All weights replicated; no collectives. Per-token attention is
local, LayerNorm is per-token, so every op is core-local.
"""

import numpy as np
import ml_dtypes

import concourse.bass as bass
import concourse.bacc as bacc
import concourse.mybir as mybir
import concourse.tile as tile
from concourse.bass_utils import run_bass_kernel_spmd
from concourse.masks import make_identity

B, S, HID = 4, 4096, 1024
H, D = 16, 64
EPS = 1e-5
NCORES = 8
TOKENS = B * S
T_CORE = TOKENS // NCORES          # 2048
P = 128
NT = T_CORE // P                   # 16 token tiles per core
KC = HID // P                      # 8 contraction chunks
SCALE = 1.0 / (D ** 0.5)

F32 = mybir.dt.float32
BF16 = mybir.dt.bfloat16
AX = mybir.AxisListType
OP = mybir.AluOpType
ACTF = mybir.ActivationFunctionType

_CACHE = {}


def _bcast_dram(ap, p=P):
    """AP that reads a [N] dram vector replicated across p partitions."""
    return bass.AP(tensor=ap.tensor, offset=ap.offset, ap=[[0, p]] + list(ap.ap))


def _build_bass(nt=NT):
    nc = bacc.Bacc("TRN2", target_bir_lowering=False, debug=False,
                   num_devices=NCORES)

    q_d = nc.dram_tensor("q", [T_CORE, HID], F32, kind="ExternalInput").ap()
    k_d = nc.dram_tensor("k", [T_CORE, HID], F32, kind="ExternalInput").ap()
    v_d = nc.dram_tensor("v", [T_CORE, HID], F32, kind="ExternalInput").ap()
    # weights pre-arranged on host to [ki=128, ko=8, n=1024] bf16
    wq_d = nc.dram_tensor("wq", [P, KC, HID], BF16, kind="ExternalInput").ap()
    wk_d = nc.dram_tensor("wk", [P, KC, HID], BF16, kind="ExternalInput").ap()
    wv_d = nc.dram_tensor("wv", [P, KC, HID], BF16, kind="ExternalInput").ap()
    wo_d = nc.dram_tensor("wo", [P, KC, HID], BF16, kind="ExternalInput").ap()
    bq_d = nc.dram_tensor("bq", [HID], F32, kind="ExternalInput").ap()
    bk_d = nc.dram_tensor("bk", [HID], F32, kind="ExternalInput").ap()
    bv_d = nc.dram_tensor("bv", [HID], F32, kind="ExternalInput").ap()
    bo_d = nc.dram_tensor("bo", [HID], F32, kind="ExternalInput").ap()
    lnw_d = nc.dram_tensor("lnw", [HID], F32, kind="ExternalInput").ap()
    lnb_d = nc.dram_tensor("lnb", [HID], F32, kind="ExternalInput").ap()
    out_d = nc.dram_tensor("out", [T_CORE, HID], F32, kind="ExternalOutput").ap()

    with tile.TileContext(nc) as tc:
        _kernel_body(nc, tc, q_d, k_d, v_d, wq_d, wk_d, wv_d, wo_d,
                     bq_d, bk_d, bv_d, bo_d, lnw_d, lnb_d, out_d, nt)
    nc.compile()
    return nc


def _kernel_body(nc, tc, q_d, k_d, v_d, wq_d, wk_d, wv_d, wo_d,
                 bq_d, bk_d, bv_d, bo_d, lnw_d, lnb_d, out_d, nt=NT):
    from contextlib import ExitStack
    ctx = ExitStack()
    with ctx:
        singles = ctx.enter_context(tc.tile_pool(name="singles", bufs=1))
        x_pool = ctx.enter_context(tc.tile_pool(name="x", bufs=2))
        xt_pool = ctx.enter_context(tc.tile_pool(name="xt", bufs=2))
        act_pool = ctx.enter_context(tc.tile_pool(name="act", bufs=2))
        tmp_pool = ctx.enter_context(tc.tile_pool(name="tmp", bufs=2))
        sm_pool = ctx.enter_context(tc.tile_pool(name="sm", bufs=2))
        y_pool = ctx.enter_context(tc.tile_pool(name="y", bufs=2))
        psum = ctx.enter_context(tc.tile_pool(name="psum", bufs=2, space="PSUM"))

        # ---- persistent constants ----
        w_sb = {}
        for name, wd in (("q", wq_d), ("k", wk_d), ("v", wv_d), ("o", wo_d)):
            t = singles.tile([P, KC, HID], BF16, tag=f"w{name}")
            nc.sync.dma_start(t[:], wd[:])
            w_sb[name] = t
        b_sb = {}
        for name, bd in (("q", bq_d), ("k", bk_d), ("v", bv_d), ("o", bo_d)):
            t = singles.tile([P, HID], F32, tag=f"b{name}")
            nc.sync.dma_start(t[:], _bcast_dram(bd))
            b_sb[name] = t
        lnw_sb = singles.tile([P, HID], F32, tag="lnw")
        nc.sync.dma_start(lnw_sb[:], _bcast_dram(lnw_d))
        lnb_sb = singles.tile([P, HID], F32, tag="lnb")
        nc.sync.dma_start(lnb_sb[:], _bcast_dram(lnb_d))

        ident_f = singles.tile([P, P], F32, tag="idf")
        make_identity(nc, ident_f)
        ident_b = singles.tile([P, P], BF16, tag="idb")
        make_identity(nc, ident_b)

        eps_sb = singles.tile([P, 1], F32, tag="eps")
        nc.vector.memset(eps_sb[:], EPS)

        for it in range(nt):
            tok = slice(it * P, (it + 1) * P)

            # ---- load + transpose q/k/v tile ----
            xts = {}
            for name, xd in (("q", q_d), ("k", k_d), ("v", v_d)):
                xt = x_pool.tile([P, HID], F32, tag="x")
                nc.sync.dma_start(xt[:], xd[tok, :])
                xT = xt_pool.tile([P, KC, P], BF16, tag=f"xT{name}")
                for c2 in range(2):
                    pt = psum.tile([P, 4 * P], F32, tag="tr")
                    for j in range(4):
                        c = c2 * 4 + j
                        nc.tensor.transpose(pt[:, j * P:(j + 1) * P],
                                            xt[:, c * P:(c + 1) * P], ident_f)
                    nc.vector.tensor_copy(
                        out=xT[:, c2 * 4:(c2 + 1) * 4, :].rearrange("p a b -> p (a b)"),
                        in_=pt[:])
                xts[name] = xT

            # ---- projections (natural layout) ----
            acts = {}
            for name in ("q", "k", "v"):
                ah = act_pool.tile([P, H, D], BF16, tag=f"h{name}")
                for half in range(2):
                    pm = psum.tile([P, 512], F32, tag="proj")
                    for c in range(KC):
                        nc.tensor.matmul(pm[:], lhsT=xts[name][:, c, :],
                                         rhs=w_sb[name][:, c, half * 512:(half + 1) * 512],
                                         start=(c == 0), stop=(c == KC - 1))
                    nc.vector.tensor_tensor(
                        out=ah.rearrange("p h d -> p (h d)")[:, half * 512:(half + 1) * 512],
                        in0=pm[:], in1=b_sb[name][:, half * 512:(half + 1) * 512],
                        op=OP.add)
                acts[name] = ah
            qh, kh, vh = acts["q"], acts["k"], acts["v"]

            # ---- scores: per-token [16,16] over head pairs ----
            # tree-sum over last axis in bf16 (2x mode) instead of 1x reduce;
            # split head groups across DVE and GPSIMD.
            def tree_sum_last(eng, pfx, src, n0, out_ap):
                cur, n = src, n0
                buf = {}
                while n > 1:
                    n //= 2
                    if n == 1:
                        nc_eng = eng
                        nc_eng.tensor_tensor(
                            out=out_ap[:, :, :, None],
                            in0=cur[:, :, :, 0:1], in1=cur[:, :, :, 1:2],
                            op=OP.add)
                        break
                    shp = list(cur.shape[:-1]) + [n]
                    dst = tmp_pool.tile(shp, BF16, tag=f"{pfx}{n}")
                    eng.tensor_tensor(out=dst[:], in0=cur[..., 0:n],
                                      in1=cur[..., n:2 * n], op=OP.add)
                    cur = dst
                return

            scores = sm_pool.tile([P, H, H], F32, tag="scores")
            for hg in range(4):
                eng = nc.vector if hg < 2 else nc.gpsimd
                hsl = slice(hg * 4, (hg + 1) * 4)
                prod = tmp_pool.tile([P, 4, H, D], BF16, tag=f"prod{hg % 2}")
                eng.tensor_tensor(
                    out=prod[:],
                    in0=qh[:, hsl, None, :].to_broadcast((P, 4, H, D)),
                    in1=kh[:, None, :, :].to_broadcast((P, 4, H, D)),
                    op=OP.mult)
                tree_sum_last(eng, f"qt{hg % 2}", prod, D, scores[:, hsl, :])

            # ---- softmax over t ----
            mx = sm_pool.tile([P, H], F32, tag="mx")
            nc.vector.tensor_reduce(out=mx[:], in_=scores[:], axis=AX.X, op=OP.max)
            ex = sm_pool.tile([P, H, H], F32, tag="ex")
            nc.vector.tensor_tensor(
                out=ex[:], in0=scores[:],
                in1=mx[:, :, None].to_broadcast((P, H, H)), op=OP.subtract)
            nc.scalar.activation(out=ex[:], in_=ex[:], func=ACTF.Exp)
            ssum = sm_pool.tile([P, H], F32, tag="ssum")
            nc.vector.tensor_reduce(out=ssum[:], in_=ex[:], axis=AX.X, op=OP.add)
            rs = sm_pool.tile([P, H], F32, tag="rs")
            nc.vector.reciprocal(out=rs[:], in_=ssum[:])
            wgt = sm_pool.tile([P, H, H], BF16, tag="wgt")
            nc.vector.tensor_tensor(
                out=wgt[:], in0=ex[:],
                in1=rs[:, :, None].to_broadcast((P, H, H)), op=OP.mult)

            # ---- attn out = w @ vh per token (DVE+GPSIMD, tree over t) ----
            vhT2 = tmp_pool.tile([P, D, H], BF16, tag="vhT2")
            nc.any.tensor_copy(out=vhT2[:], in_=vh.rearrange("p t d -> p d t"))
            attn_f = tmp_pool.tile([P, H, D], F32, tag="attnf")
            for hg in range(4):
                eng = nc.vector if hg < 2 else nc.gpsimd
                hsl = slice(hg * 4, (hg + 1) * 4)
                prod2 = tmp_pool.tile([P, 4, D, H], BF16, tag=f"prod{hg % 2}")
                eng.tensor_tensor(
                    out=prod2[:],
                    in0=wgt[:, hsl, None, :].to_broadcast((P, 4, D, H)),
                    in1=vhT2[:, None, :, :].to_broadcast((P, 4, D, H)),
                    op=OP.mult)
                tree_sum_last(eng, f"at{hg % 2}", prod2, H, attn_f[:, hsl, :])

            attn_b = tmp_pool.tile([P, H * D], BF16, tag="attnb")
            nc.any.tensor_copy(out=attn_b[:],
                               in_=attn_f.rearrange("p h d -> p (h d)"))

            # ---- transpose attn, output projection ----
            aT = xt_pool.tile([P, KC, P], BF16, tag="aT")
            for c2 in range(2):
                pt = psum.tile([P, 4 * P], BF16, tag="trb")
                for j in range(4):
                    c = c2 * 4 + j
                    nc.tensor.transpose(pt[:, j * P:(j + 1) * P],
                                        attn_b[:, c * P:(c + 1) * P], ident_b)
                nc.vector.tensor_copy(
                    out=aT[:, c2 * 4:(c2 + 1) * 4, :].rearrange("p a b -> p (a b)"),
                    in_=pt[:])

            y = y_pool.tile([P, HID], F32, tag="y")
            for half in range(2):
                pm = psum.tile([P, 512], F32, tag="proj")
                for c in range(KC):
                    nc.tensor.matmul(pm[:], lhsT=aT[:, c, :],
                                     rhs=w_sb["o"][:, c, half * 512:(half + 1) * 512],
                                     start=(c == 0), stop=(c == KC - 1))
                nc.vector.tensor_tensor(
                    out=y[:, half * 512:(half + 1) * 512],
                    in0=pm[:], in1=b_sb["o"][:, half * 512:(half + 1) * 512],
                    op=OP.add)

            # ---- LayerNorm-ish: ln_w*(y-mu)/(var_ddof1+eps) + ln_b ----
            stats = sm_pool.tile([P, 2, nc.vector.BN_STATS_DIM], F32, tag="bns")
            yg = y.rearrange("p (a b) -> p a b", a=2)
            for sg in range(2):
                nc.vector.bn_stats(out=stats[:, sg, :], in_=yg[:, sg, :])
            mv = sm_pool.tile([P, 2], F32, tag="mv")
            nc.vector.bn_aggr(out=mv[:], in_=stats[:])
            rvar = sm_pool.tile([P, 1], F32, tag="rvar")
            nc.vector.tensor_scalar(
                out=rvar[:], in0=mv[:, 1:2],
                scalar1=float(HID) / float(HID - 1), scalar2=EPS,
                op0=OP.mult, op1=OP.add)
            nc.vector.reciprocal(out=rvar[:], in_=rvar[:])
            xc = y_pool.tile([P, HID], F32, tag="xc")
            nc.vector.tensor_scalar(
                out=xc[:], in0=y[:], scalar1=mv[:, 0:1], scalar2=rvar[:],
                op0=OP.subtract, op1=OP.mult)
            o_sb = y_pool.tile([P, HID], F32, tag="osb")
            nc.vector.tensor_tensor(out=xc[:], in0=xc[:], in1=lnw_sb[:], op=OP.mult)
            nc.vector.tensor_tensor(out=o_sb[:], in0=xc[:], in1=lnb_sb[:], op=OP.add)

            nc.sync.dma_start(out_d[tok, :], o_sb[:])


def _prep_host(Wq, bq, Wk, bk, Wv, bv, Wo, bo):
    """Host-side weight prep: fold score scale into Wq/bq; cast to bf16 and
    rearrange [K,N] -> [ki, ko, n]."""
    def rearr(w):
        return np.ascontiguousarray(
            np.asarray(w, dtype=np.float32).reshape(KC, P, HID).transpose(1, 0, 2)
        ).astype(ml_dtypes.bfloat16)
    wq = rearr(np.asarray(Wq, np.float32) * SCALE)
    wk = rearr(Wk)
    wv = rearr(Wv)
    wo = rearr(Wo)
    return {
        "wq": wq, "wk": wk, "wv": wv, "wo": wo,
        "bq": np.asarray(bq, np.float32) * np.float32(SCALE),
        "bk": np.asarray(bk, np.float32),
        "bv": np.asarray(bv, np.float32),
        "bo": np.asarray(bo, np.float32),
    }


def get_nc(nt=NT):
    key = f"nc{nt}"
    if key not in _CACHE:
        _CACHE[key] = _build_bass(nt)
    return _CACHE[key]


def make_in_maps(q, k, v, Wq, bq, Wk, bk, Wv, bv, Wo, bo, ln_w, ln_b):
    shared = _prep_host(Wq, bq, Wk, bk, Wv, bv, Wo, bo)
    shared["lnw"] = np.asarray(ln_w, np.float32)
    shared["lnb"] = np.asarray(ln_b, np.float32)
    qf = np.asarray(q, np.float32).reshape(TOKENS, HID)
    kf = np.asarray(k, np.float32).reshape(TOKENS, HID)
    vf = np.asarray(v, np.float32).reshape(TOKENS, HID)
    in_maps = []
    for c in range(NCORES):
        sl = slice(c * T_CORE, (c + 1) * T_CORE)
        m = dict(shared)
        m["q"] = np.ascontiguousarray(qf[sl])
        m["k"] = np.ascontiguousarray(kf[sl])
        m["v"] = np.ascontiguousarray(vf[sl])
        in_maps.append(m)
    return in_maps


def kernel(q, k, v, Wq, bq, Wk, bk, Wv, bv, Wo, bo, ln_w, ln_b):
    nc = get_nc()
    in_maps = make_in_maps(q, k, v, Wq, bq, Wk, bk, Wv, bv, Wo, bo, ln_w, ln_b)
    res = run_bass_kernel_spmd(nc, in_maps, list(range(NCORES))).results
    out = np.concatenate([res[c]["out"] for c in range(NCORES)], axis=0)
    return out.reshape(B, S, HID).astype(np.float32)


# revision 19
# speedup vs baseline: 2.2029x; 2.2029x over previous
"""Trainium2 Bass kernel for nn_MultiHeadAttention_21345987461791.

Reference computation (B=4, S=4096, HID=1024, NUM_HEADS=16, HEAD_DIM=64):
    qh = (q @ Wq + bq)  -> [B,S,16,64]   (same for k, v)
    scores = einsum('bshd,bstd->bsht', qh, kh) / 8     # per-token [16,16]
    w = softmax(scores, -1)
    out = einsum('bsht,bstd->bshd', w, vh) @ Wo + bo
    LayerNorm-ish: ln_w * (out - mu) / (var_unbiased + eps) + ln_b   (NO sqrt!)

Sharding: pure data-parallel over tokens. B*S = 16384 tokens -> 2048 per core
(8 cores). All weights replicated; no collectives. Per-token attention is
local, LayerNorm is per-token, so every op is core-local.
"""

import os
import numpy as np
import ml_dtypes

import concourse.bass as bass
import concourse.bacc as bacc
import concourse.mybir as mybir
import concourse.tile as tile
from concourse.bass_utils import run_bass_kernel_spmd
from concourse.masks import make_identity

B, S, HID = 4, 4096, 1024
H, D = 16, 64
EPS = 1e-5
NCORES = 8
TOKENS = B * S
T_CORE = TOKENS // NCORES          # 2048
P = 128
NT = T_CORE // P                   # 16 token tiles per core
KC = HID // P                      # 8 contraction chunks
SCALE = 1.0 / (D ** 0.5)

F32 = mybir.dt.float32
BF16 = mybir.dt.bfloat16
AX = mybir.AxisListType
OP = mybir.AluOpType
ACTF = mybir.ActivationFunctionType

_CACHE = {}
GP_SPLIT = os.environ.get("K_GP_SPLIT", "0") == "1"
X_BUFS = int(os.environ.get("K_X_BUFS", "3"))


def _bcast_dram(ap, p=P):
    """AP that reads a [N] dram vector replicated across p partitions."""
    return bass.AP(tensor=ap.tensor, offset=ap.offset, ap=[[0, p]] + list(ap.ap))


def _build_bass(nt=NT, reps=1):
    nc = bacc.Bacc("TRN2", target_bir_lowering=False, debug=False,
                   num_devices=NCORES)

    q_d = nc.dram_tensor("q", [T_CORE, HID], F32, kind="ExternalInput").ap()
    k_d = nc.dram_tensor("k", [T_CORE, HID], F32, kind="ExternalInput").ap()
    v_d = nc.dram_tensor("v", [T_CORE, HID], F32, kind="ExternalInput").ap()
    # weights pre-arranged on host to [ki=128, ko=8, n=1024] bf16
    wq_d = nc.dram_tensor("wq", [P, KC, HID], BF16, kind="ExternalInput").ap()
    wk_d = nc.dram_tensor("wk", [P, KC, HID], BF16, kind="ExternalInput").ap()
    wv_d = nc.dram_tensor("wv", [P, KC, HID], BF16, kind="ExternalInput").ap()
    wo_d = nc.dram_tensor("wo", [P, KC, HID], BF16, kind="ExternalInput").ap()
    bq_d = nc.dram_tensor("bq", [HID], BF16, kind="ExternalInput").ap()
    bk_d = nc.dram_tensor("bk", [HID], BF16, kind="ExternalInput").ap()
    bv_d = nc.dram_tensor("bv", [HID], BF16, kind="ExternalInput").ap()
    bo_d = nc.dram_tensor("bo", [HID], BF16, kind="ExternalInput").ap()
    lnw_d = nc.dram_tensor("lnw", [HID], BF16, kind="ExternalInput").ap()
    lnb_d = nc.dram_tensor("lnb", [HID], BF16, kind="ExternalInput").ap()
    out_d = nc.dram_tensor("out", [T_CORE, HID], F32, kind="ExternalOutput").ap()

    with tile.TileContext(nc) as tc:
        _kernel_body(nc, tc, q_d, k_d, v_d, wq_d, wk_d, wv_d, wo_d,
                     bq_d, bk_d, bv_d, bo_d, lnw_d, lnb_d, out_d, nt, reps)
    nc.compile()
    return nc


def _kernel_body(nc, tc, q_d, k_d, v_d, wq_d, wk_d, wv_d, wo_d,
                 bq_d, bk_d, bv_d, bo_d, lnw_d, lnb_d, out_d, nt=NT, reps=1):
    from contextlib import ExitStack
    ctx = ExitStack()
    with ctx:
        singles = ctx.enter_context(tc.tile_pool(name="singles", bufs=1))
        x_pool = ctx.enter_context(tc.tile_pool(name="x", bufs=X_BUFS))
        xt_pool = ctx.enter_context(tc.tile_pool(name="xt", bufs=2))
        act_pool = ctx.enter_context(tc.tile_pool(name="act", bufs=2))
        tmp_pool = ctx.enter_context(tc.tile_pool(name="tmp", bufs=2))
        sm_pool = ctx.enter_context(tc.tile_pool(name="sm", bufs=2))
        y_pool = ctx.enter_context(tc.tile_pool(name="y", bufs=2))
        psum = ctx.enter_context(tc.tile_pool(name="psum", bufs=2, space="PSUM"))

        # ---- persistent constants ----
        w_sb = {}
        for name, wd in (("q", wq_d), ("k", wk_d), ("v", wv_d), ("o", wo_d)):
            t = singles.tile([P, KC, HID], BF16, tag=f"w{name}")
            nc.sync.dma_start(t[:], wd[:])
            w_sb[name] = t
        b_sb = {}
        for name, bd in (("q", bq_d), ("k", bk_d), ("v", bv_d), ("o", bo_d)):
            t = singles.tile([P, HID], BF16, tag=f"b{name}")
            nc.sync.dma_start(t[:], _bcast_dram(bd))
            b_sb[name] = t
        lnw_sb = singles.tile([P, HID], BF16, tag="lnw")
        nc.sync.dma_start(lnw_sb[:], _bcast_dram(lnw_d))
        lnb_sb = singles.tile([P, HID], BF16, tag="lnb")
        nc.sync.dma_start(lnb_sb[:], _bcast_dram(lnb_d))

        ident_f = singles.tile([P, P], F32, tag="idf")
        make_identity(nc, ident_f)
        ident_b = singles.tile([P, P], BF16, tag="idb")
        make_identity(nc, ident_b)

        eps_sb = singles.tile([P, 1], F32, tag="eps")
        nc.vector.memset(eps_sb[:], EPS)

        for it in [t for _ in range(reps) for t in range(nt)]:
            tok = slice(it * P, (it + 1) * P)

            # ---- load + transpose q/k/v tile ----
            xts = {}
            for name, xd in (("q", q_d), ("k", k_d), ("v", v_d)):
                xt = x_pool.tile([P, HID], F32, tag="x")
                nc.sync.dma_start(xt[:], xd[tok, :])
                xT = xt_pool.tile([P, KC, P], BF16, tag=f"xT{name}")
                for c2 in range(2):
                    pt = psum.tile([P, 4 * P], F32, tag="tr")
                    for j in range(4):
                        c = c2 * 4 + j
                        nc.tensor.transpose(pt[:, j * P:(j + 1) * P],
                                            xt[:, c * P:(c + 1) * P], ident_f)
                    nc.any.tensor_copy(
                        out=xT[:, c2 * 4:(c2 + 1) * 4, :].rearrange("p a b -> p (a b)"),
                        in_=pt[:])
                xts[name] = xT

            # ---- projections (natural layout) ----
            acts = {}
            for name in ("q", "k", "v"):
                ah = act_pool.tile([P, H, D], BF16, tag=f"h{name}")
                for half in range(2):
                    pm = psum.tile([P, 512], F32, tag="proj")
                    for c in range(KC):
                        nc.tensor.matmul(pm[:], lhsT=xts[name][:, c, :],
                                         rhs=w_sb[name][:, c, half * 512:(half + 1) * 512],
                                         start=(c == 0), stop=(c == KC - 1))
                    nc.any.tensor_tensor(
                        out=ah.rearrange("p h d -> p (h d)")[:, half * 512:(half + 1) * 512],
                        in0=pm[:], in1=b_sb[name][:, half * 512:(half + 1) * 512],
                        op=OP.add)
                acts[name] = ah
            qh, kh, vh = acts["q"], acts["k"], acts["v"]

            # ---- scores: per-token [16,16] over head pairs ----
            # tree-sum over last axis in bf16 (2x mode) instead of 1x reduce;
            # split head groups across DVE and GPSIMD.
            def tree_sum_last(eng, pfx, src, n0, out_ap):
                # slots with one shared tag rotate via pool bufs (2)
                cur, n = src, n0
                maxshape = list(src.shape[:-1]) + [n0 // 2]
                while n > 1:
                    n //= 2
                    if n == 1:
                        eng.tensor_tensor(
                            out=out_ap[:, :, :, None],
                            in0=cur[:, :, :, 0:1], in1=cur[:, :, :, 1:2],
                            op=OP.add)
                        break
                    dst = tmp_pool.tile(maxshape, BF16, tag=pfx)
                    dst = dst[..., :n]
                    eng.tensor_tensor(out=dst[:], in0=cur[..., 0:n],
                                      in1=cur[..., n:2 * n], op=OP.add)
                    cur = dst
                return

            scores = sm_pool.tile([P, H, H], F32, tag="scores")
            for hg in range(2):
                eng = nc.vector if (hg < 1 or not GP_SPLIT) else nc.gpsimd
                hsl = slice(hg * 8, (hg + 1) * 8)
                prod = tmp_pool.tile([P, 8, H, D], BF16, tag="prod")
                eng.tensor_tensor(
                    out=prod[:],
                    in0=qh[:, hsl, None, :].to_broadcast((P, 8, H, D)),
                    in1=kh[:, None, :, :].to_broadcast((P, 8, H, D)),
                    op=OP.mult)
                tree_sum_last(eng, "tree", prod, D, scores[:, hsl, :])

            # ---- softmax over t ----
            nc.scalar.activation(out=scores[:], in_=scores[:], func=ACTF.Exp)
            ssum = sm_pool.tile([P, H], F32, tag="ssum")
            nc.vector.tensor_reduce(out=ssum[:], in_=scores[:], axis=AX.X, op=OP.add)
            rs = sm_pool.tile([P, H], F32, tag="rs")
            nc.vector.reciprocal(out=rs[:], in_=ssum[:])
            wgt = sm_pool.tile([P, H, H], BF16, tag="wgt")
            nc.vector.tensor_tensor(
                out=wgt[:], in0=scores[:],
                in1=rs[:, :, None].to_broadcast((P, H, H)), op=OP.mult)

            # ---- attn out = w @ vh per token (DVE+GPSIMD, tree over t) ----
            vhT2 = tmp_pool.tile([P, D, H], BF16, tag="vhT2")
            nc.any.tensor_copy(out=vhT2[:], in_=vh.rearrange("p t d -> p d t"))
            attn_b = tmp_pool.tile([P, H * D], BF16, tag="attnb")
            attn_v = attn_b.rearrange("p (h d) -> p h d", h=H)
            for hg in range(2):
                eng = nc.vector if (hg < 1 or not GP_SPLIT) else nc.gpsimd
                hsl = slice(hg * 8, (hg + 1) * 8)
                prod2 = tmp_pool.tile([P, 8, D, H], BF16, tag="prod")
                eng.tensor_tensor(
                    out=prod2[:],
                    in0=wgt[:, hsl, None, :].to_broadcast((P, 8, D, H)),
                    in1=vhT2[:, None, :, :].to_broadcast((P, 8, D, H)),
                    op=OP.mult)
                tree_sum_last(eng, "tree", prod2, H, attn_v[:, hsl, :])

            # ---- transpose attn, output projection ----
            aT = xt_pool.tile([P, KC, P], BF16, tag="aT")
            for c2 in range(2):
                pt = psum.tile([P, 4 * P], BF16, tag="trb")
                for j in range(4):
                    c = c2 * 4 + j
                    nc.tensor.transpose(pt[:, j * P:(j + 1) * P],
                                        attn_b[:, c * P:(c + 1) * P], ident_b)
                nc.any.tensor_copy(
                    out=aT[:, c2 * 4:(c2 + 1) * 4, :].rearrange("p a b -> p (a b)"),
                    in_=pt[:])

            y = y_pool.tile([P, HID], F32, tag="y")
            for half in range(2):
                pm = psum.tile([P, 512], F32, tag="proj")
                for c in range(KC):
                    nc.tensor.matmul(pm[:], lhsT=aT[:, c, :],
                                     rhs=w_sb["o"][:, c, half * 512:(half + 1) * 512],
                                     start=(c == 0), stop=(c == KC - 1))
                nc.any.tensor_tensor(
                    out=y[:, half * 512:(half + 1) * 512],
                    in0=pm[:], in1=b_sb["o"][:, half * 512:(half + 1) * 512],
                    op=OP.add)

            # ---- LayerNorm-ish: ln_w*(y-mu)/(var_ddof1+eps) + ln_b ----
            stats = sm_pool.tile([P, 2, nc.vector.BN_STATS_DIM], F32, tag="bns")
            yg = y.rearrange("p (a b) -> p a b", a=2)
            for sg in range(2):
                nc.vector.bn_stats(out=stats[:, sg, :], in_=yg[:, sg, :])
            mv = sm_pool.tile([P, 2], F32, tag="mv")
            nc.vector.bn_aggr(out=mv[:], in_=stats[:])
            rvar = sm_pool.tile([P, 1], F32, tag="rvar")
            nc.vector.tensor_scalar(
                out=rvar[:], in0=mv[:, 1:2],
                scalar1=float(HID) / float(HID - 1), scalar2=EPS,
                op0=OP.mult, op1=OP.add)
            nc.vector.reciprocal(out=rvar[:], in_=rvar[:])
            nc.vector.tensor_scalar(
                out=y[:], in0=y[:], scalar1=mv[:, 0:1], scalar2=rvar[:],
                op0=OP.subtract, op1=OP.mult)
            nc.vector.tensor_tensor(out=y[:], in0=y[:], in1=lnw_sb[:], op=OP.mult)
            nc.vector.tensor_tensor(out=y[:], in0=y[:], in1=lnb_sb[:], op=OP.add)

            nc.sync.dma_start(out_d[tok, :], y[:])


def _prep_host(Wq, bq, Wk, bk, Wv, bv, Wo, bo):
    """Host-side weight prep: fold score scale into Wq/bq; cast to bf16 and
    rearrange [K,N] -> [ki, ko, n]."""
    def rearr(w):
        return np.ascontiguousarray(
            np.asarray(w, dtype=np.float32).reshape(KC, P, HID).transpose(1, 0, 2)
        ).astype(ml_dtypes.bfloat16)
    wq = rearr(np.asarray(Wq, np.float32) * SCALE)
    wk = rearr(Wk)
    wv = rearr(Wv)
    wo = rearr(Wo)
    return {
        "wq": wq, "wk": wk, "wv": wv, "wo": wo,
        "bq": (np.asarray(bq, np.float32) * np.float32(SCALE)).astype(ml_dtypes.bfloat16),
        "bk": np.asarray(bk, np.float32).astype(ml_dtypes.bfloat16),
        "bv": np.asarray(bv, np.float32).astype(ml_dtypes.bfloat16),
        "bo": np.asarray(bo, np.float32).astype(ml_dtypes.bfloat16),
    }


def get_nc(nt=NT, reps=1):
    key = f"nc{nt}_{reps}"
    if key not in _CACHE:
        _CACHE[key] = _build_bass(nt, reps)
    return _CACHE[key]


def make_in_maps(q, k, v, Wq, bq, Wk, bk, Wv, bv, Wo, bo, ln_w, ln_b):
    shared = _prep_host(Wq, bq, Wk, bk, Wv, bv, Wo, bo)
    shared["lnw"] = np.asarray(ln_w, np.float32).astype(ml_dtypes.bfloat16)
    shared["lnb"] = np.asarray(ln_b, np.float32).astype(ml_dtypes.bfloat16)
    qf = np.asarray(q, np.float32).reshape(TOKENS, HID)
    kf = np.asarray(k, np.float32).reshape(TOKENS, HID)
    vf = np.asarray(v, np.float32).reshape(TOKENS, HID)
    in_maps = []
    for c in range(NCORES):
        sl = slice(c * T_CORE, (c + 1) * T_CORE)
        m = dict(shared)
        m["q"] = np.ascontiguousarray(qf[sl])
        m["k"] = np.ascontiguousarray(kf[sl])
        m["v"] = np.ascontiguousarray(vf[sl])
        in_maps.append(m)
    return in_maps


def kernel(q, k, v, Wq, bq, Wk, bk, Wv, bv, Wo, bo, ln_w, ln_b):
    nc = get_nc()
    in_maps = make_in_maps(q, k, v, Wq, bq, Wk, bk, Wv, bv, Wo, bo, ln_w, ln_b)
    res = run_bass_kernel_spmd(nc, in_maps, list(range(NCORES))).results
    out = np.concatenate([res[c]["out"] for c in range(NCORES)], axis=0)
    return out.reshape(B, S, HID).astype(np.float32)
